# revision 1
# baseline (speedup 1.0000x reference)
# GQA attention block on 8 Trainium2 NeuronCores.
# Sharding: core = (batch b in {0,1}) x (tensor-parallel t in {0..3}).
# Each core: batch row b, 4 query heads {4t..4t+3}, 2 kv heads {2t, 2t+1}.
# W_Q/W_K/W_V split column-wise (per-head), W_O row-wise; the 4 TP partial
# outputs per batch are summed on the host (the "all-reduce").
import math
import sys

sys.path.insert(0, "/opt/trn_rl_repo")

import ml_dtypes
import numpy as np

import concourse.bacc as bacc
import concourse.bass as bass
import concourse.mybir as mybir
import concourse.tile as tile
from contextlib import ExitStack

BF = mybir.dt.bfloat16
F32 = mybir.dt.float32
bfnp = ml_dtypes.bfloat16

EMB = 2048
HEADS = 16
G = 2
HD = 128          # head dim
KV = HEADS // G   # 8 kv heads
B = 2
S = 2048
NCORES = 8
TP = 4
HQ = HEADS // TP       # 4 q heads per core
HKV = KV // TP         # 2 kv heads per core
NE = EMB // 128        # 16 contraction chunks
SC4 = S // 512         # 4 s-chunks of 512
SC16 = S // 128        # 16 s-chunks of 128
SCALE = 1.0 / math.sqrt(float(EMB))

_NC = None


def _build_program(loop_n=None):
    nc = bacc.Bacc("TRN2", target_bir_lowering=False, debug=False)

    xT = nc.dram_tensor("xT", (EMB, S), BF, kind="ExternalInput")
    wq = nc.dram_tensor("wq", (EMB, HQ * HD), BF, kind="ExternalInput")
    wk = nc.dram_tensor("wk", (EMB, HKV * HD), BF, kind="ExternalInput")
    wv = nc.dram_tensor("wv", (EMB, HKV * HD), BF, kind="ExternalInput")
    wo = nc.dram_tensor("wo", (HQ * HD, EMB), BF, kind="ExternalInput")
    cosT = nc.dram_tensor("cosT", (HD, S), F32, kind="ExternalInput")
    sinT = nc.dram_tensor("sinT", (HD, S), F32, kind="ExternalInput")
    out = nc.dram_tensor("out", (S, EMB), F32, kind="ExternalOutput")

    with tile.TileContext(nc) as tc, ExitStack() as ctx:
        persist = ctx.enter_context(tc.tile_pool(name="persist", bufs=1))
        # qk_sb j-blocks: 0..3 = roped Q heads, 4..5 = roped K kv-heads; [d, s]
        qk_sb = persist.tile([128, HQ + HKV, S], BF)
        # V in [t, d] layout: [t_part, t_chunk, kvl*128+d]
        v_sb = persist.tile([128, SC16, HKV * HD], BF)
        ctx_sb = persist.tile([128, HQ, S], BF)      # [d, head, s]
        wo_sb = persist.tile([128, HQ, EMB], BF)     # [d, head, e_out]
        ones_sb = persist.tile([128, 1], BF)
        nc.vector.memset(ones_sb, 1.0)
        for jb in range(HQ):
            nc.sync.dma_start(out=wo_sb[:, jb, :], in_=wo[jb * 128:(jb + 1) * 128, :])

        def _phases():
            # ---------------- Phase 1: projections + RoPE ----------------
            with tc.tile_pool(name="xt", bufs=1) as xt_pool, \
                 tc.tile_pool(name="wts", bufs=1) as w_pool, \
                 tc.tile_pool(name="ropet", bufs=4) as rope_t, \
                 tc.tile_pool(name="pproj", bufs=8, space=bass.MemorySpace.PSUM) as pp:
                xt_sb = xt_pool.tile([128, NE, S], BF)
                for c in range(NE):
                    nc.sync.dma_start(out=xt_sb[:, c, :], in_=xT[c * 128:(c + 1) * 128, :])
                wq_sb = w_pool.tile([128, NE, HQ * HD], BF)
                wk_sb = w_pool.tile([128, NE, HKV * HD], BF)
                wv_sb = w_pool.tile([128, NE, HKV * HD], BF)
                cos_sb = w_pool.tile([128, S], F32)
                sin_sb = w_pool.tile([128, S], F32)
                for c in range(NE):
                    nc.sync.dma_start(out=wq_sb[:, c, :], in_=wq[c * 128:(c + 1) * 128, :])
                    nc.sync.dma_start(out=wk_sb[:, c, :], in_=wk[c * 128:(c + 1) * 128, :])
                    nc.sync.dma_start(out=wv_sb[:, c, :], in_=wv[c * 128:(c + 1) * 128, :])
                nc.sync.dma_start(out=cos_sb, in_=cosT[:, :])
                nc.sync.dma_start(out=sin_sb, in_=sinT[:, :])

                # Q/K projection in transposed [d, s] layout + RoPE
                def do_qk(jb):
                    pts = []
                    for sc in range(SC4):
                        pts.append(pp.tile([128, 512], F32, tag="pts", name=f"pts_{jb}_{sc}"))
                    for c in range(NE):
                        if jb < HQ:
                            lhsT = wq_sb[:, c, jb * 128:(jb + 1) * 128]
                        else:
                            kvl = jb - HQ
                            lhsT = wk_sb[:, c, kvl * 128:(kvl + 1) * 128]
                        for sc in range(SC4):
                            nc.tensor.matmul(
                                pts[sc], lhsT, xt_sb[:, c, sc * 512:(sc + 1) * 512],
                                start=(c == 0), stop=(c == NE - 1),
                            )
                    for sc in range(SC4):
                        sl = slice(sc * 512, (sc + 1) * 512)
                        xs = rope_t.tile([128, 512], F32, tag="xs")
                        nc.scalar.copy(xs, pts[sc])
                        xw = rope_t.tile([128, 512], F32, tag="xw")
                        nc.sync.dma_start(out=xw[0:64, :], in_=xs[64:128, :])
                        nc.sync.dma_start(out=xw[64:128, :], in_=xs[0:64, :])
                        t1 = rope_t.tile([128, 512], F32, tag="t1")
                        nc.vector.tensor_mul(t1, xs, cos_sb[:, sl])
                        nc.vector.tensor_mul(xw, xw, sin_sb[:, sl])
                        nc.vector.tensor_add(qk_sb[:, jb, sl], t1, xw)

                # V in [t, d] layout (no rope): out[t=128, kvl*128+d]
                def do_v():
                    for st in range(SC16):
                        pv = pp.tile([128, 512], F32, tag="pts", name=f"pv_{st}")
                        for c in range(NE):
                            nc.tensor.matmul(
                                pv[:, 0:HKV * HD],
                                xt_sb[:, c, st * 128:(st + 1) * 128],
                                wv_sb[:, c, :],
                                start=(c == 0), stop=(c == NE - 1),
                            )
                        nc.scalar.copy(v_sb[:, st, :], pv[:, 0:HKV * HD])

                # K and V first so attention h=0 can begin while Q1..3 project
                do_qk(HQ)
                do_qk(HQ + 1)
                do_v()
                for jb in range(HQ):
                    do_qk(jb)

            # ---------------- Phase 2: attention ----------------
            with tc.tile_pool(name="pscore", bufs=3, space=bass.MemorySpace.PSUM) as psc, \
                 tc.tile_pool(name="pctx", bufs=2, space=bass.MemorySpace.PSUM) as pcx, \
                 tc.tile_pool(name="pden", bufs=2, space=bass.MemorySpace.PSUM) as pdn, \
                 tc.tile_pool(name="expp", bufs=6) as expp, \
                 tc.tile_pool(name="misc", bufs=2) as misc:
                for h in range(HQ):
                    kvjb = HQ + h // 2     # K block in qk_sb
                    kvl = h // 2           # local kv index into v_sb columns
                    for sc in range(SC4):
                        ssl = slice(sc * 512, (sc + 1) * 512)
                        cps = pcx.tile([128, 512], F32, tag="cps")
                        dps = pdn.tile([1, 512], F32, tag="dps")
                        for tcn in range(SC16):
                            sps = psc.tile([128, 512], F32, tag="sps")
                            nc.tensor.matmul(
                                sps,
                                qk_sb[:, kvjb, tcn * 128:(tcn + 1) * 128],
                                qk_sb[:, h, ssl],
                                start=True, stop=True,
                            )
                            ex = expp.tile([128, 512], BF, tag="ex")
                            nc.scalar.activation(
                                ex, sps, mybir.ActivationFunctionType.Exp, scale=SCALE
                            )
                            nc.tensor.matmul(
                                cps,
                                v_sb[:, tcn, kvl * 128:(kvl + 1) * 128],
                                ex,
                                start=(tcn == 0), stop=(tcn == SC16 - 1),
                            )
                            nc.tensor.matmul(
                                dps, ones_sb, ex,
                                start=(tcn == 0), stop=(tcn == SC16 - 1),
                            )
                        rc = misc.tile([1, 512], F32, tag="rc")
                        nc.vector.reciprocal(rc, dps)
                        rb = misc.tile([128, 512], F32, tag="rb")
                        nc.gpsimd.partition_broadcast(rb, rc)
                        nc.vector.tensor_mul(ctx_sb[:, h, ssl], cps, rb)

            # ---------------- Phase 3: output projection ----------------
            with tc.tile_pool(name="pout", bufs=4, space=bass.MemorySpace.PSUM) as pou, \
                 tc.tile_pool(name="outs", bufs=4) as outp:
                for so in range(SC16):
                    for ec in range(SC4):
                        ops = pou.tile([128, 512], F32, tag="ops")
                        for hl in range(HQ):
                            nc.tensor.matmul(
                                ops,
                                ctx_sb[:, hl, so * 128:(so + 1) * 128],
                                wo_sb[:, hl, ec * 512:(ec + 1) * 512],
                                start=(hl == 0), stop=(hl == HQ - 1),
                            )
                        ot = outp.tile([128, 512], F32, tag="ot")
                        nc.scalar.copy(ot, ops)
                        nc.sync.dma_start(
                            out=out[so * 128:(so + 1) * 128, ec * 512:(ec + 1) * 512],
                            in_=ot,
                        )


        if loop_n is not None:
            with tc.For_i(0, loop_n, 1):
                _phases()
        else:
            _phases()

    nc.compile()
    return nc


def _get_nc():
    global _NC
    if _NC is None:
        _NC = _build_program()
    return _NC


def _rope_tables():
    half = HD // 2
    inv_freq = 1.0 / (10000.0 ** (np.arange(half, dtype=np.float64) * 2.0 / HD))
    ang = np.arange(S, dtype=np.float64)[:, None] * inv_freq[None, :]  # (S, 64)
    cos = np.concatenate([np.cos(ang), np.cos(ang)], axis=1).T  # (128, S)
    sin = np.concatenate([-np.sin(ang), np.sin(ang)], axis=1).T  # pre-signed
    return (np.ascontiguousarray(cos, dtype=np.float32),
            np.ascontiguousarray(sin, dtype=np.float32))


def build_in_maps(x, W_Q, W_K, W_V, W_O):
    x = np.asarray(x, dtype=np.float32)
    W_Q = np.asarray(W_Q, dtype=np.float32)
    W_K = np.asarray(W_K, dtype=np.float32)
    W_V = np.asarray(W_V, dtype=np.float32)
    W_O = np.asarray(W_O, dtype=np.float32)
    cos, sin = _rope_tables()
    in_maps = []
    xTb = [np.ascontiguousarray(x[b].T).astype(bfnp) for b in range(B)]
    for b in range(B):
        for t in range(TP):
            qheads = list(range(HQ * t, HQ * t + HQ))
            kvheads = [HKV * t + i for i in range(HKV)]
            idxq = [d * HEADS + h for h in qheads for d in range(HD)]
            idxkv = [d * KV + kv for kv in kvheads for d in range(HD)]
            rows_o = [h * HD + d for h in qheads for d in range(HD)]
            in_maps.append(dict(
                xT=xTb[b],
                wq=np.ascontiguousarray(W_Q[idxq, :].T).astype(bfnp),
                wk=np.ascontiguousarray(W_K[idxkv, :].T).astype(bfnp),
                wv=np.ascontiguousarray(W_V[idxkv, :].T).astype(bfnp),
                wo=np.ascontiguousarray(W_O[:, rows_o].T).astype(bfnp),
                cosT=cos,
                sinT=sin,
            ))
    return in_maps


def combine_outs(outs):
    out = np.empty((B, S, EMB), dtype=np.float32)
    for b in range(B):
        acc = outs[TP * b].astype(np.float32).copy()
        for t in range(1, TP):
            acc += outs[TP * b + t]
        out[b] = acc
    return out


LAST_RESULTS = None


def kernel(x, W_Q, W_K, W_V, W_O):
    global LAST_RESULTS
    from concourse.bass_utils import run_bass_kernel_spmd

    nc = _get_nc()
    in_maps = build_in_maps(x, W_Q, W_K, W_V, W_O)
    res = run_bass_kernel_spmd(nc, in_maps, list(range(NCORES)))
    LAST_RESULTS = res
    outs = [r["out"] for r in res.results]
    return combine_outs(outs)



# revision 12
# speedup vs baseline: 10.1127x; 10.1127x over previous
# GQA attention block on 8 Trainium2 NeuronCores.
# Sharding: core = (batch b in {0,1}) x (tensor-parallel t in {0..3}).
# Each core: batch row b, 4 query heads {4t..4t+3}, 2 kv heads {2t, 2t+1}.
# W_Q/W_K/W_V split column-wise (per-head), W_O row-wise; the 4 TP partial
# outputs per batch are summed on the host (the "all-reduce").
#
# Schedule (per core): prefix = projections + RoPE (PE-dense, DMA-ordered so
# K-proj starts ~4us in); main = 16 groups (sc-outer, h-inner) software-
# pipelined one group deep: sps(g) | exp(g) on ACT | cps(g-1) | out-proj
# filler of window sc-1; softmax denominator via DVE bf16 tree + gpsimd
# partition_all_reduce (no PE matmuls wasted on the ones-reduction).
import math
import sys

sys.path.insert(0, "/opt/trn_rl_repo")

import ml_dtypes
import numpy as np

import concourse.bacc as bacc
import concourse.bass as bass
import concourse.bass_isa as bass_isa
import concourse.mybir as mybir
import concourse.tile as tile
from contextlib import ExitStack

BF = mybir.dt.bfloat16
F32 = mybir.dt.float32
bfnp = ml_dtypes.bfloat16

EMB = 2048
HEADS = 16
G = 2
HD = 128          # head dim
KV = HEADS // G   # 8 kv heads
B = 2
S = 2048
NCORES = 8
TP = 4
HQ = HEADS // TP       # 4 q heads per core
HKV = KV // TP         # 2 kv heads per core
NE = EMB // 128        # 16 contraction chunks
SC4 = S // 512         # 4 s-chunks of 512
SC16 = S // 128        # 16 s-chunks of 128
NPAIR = SC16 // 2      # 8 t-chunk pairs in attention
SCALE = 1.0 / math.sqrt(float(EMB))

_NC = None


def _build_program(loop_n=None, sps_bufs=2, pou_bufs=2):
    nc = bacc.Bacc("TRN2", target_bir_lowering=False, debug=False)

    xT = nc.dram_tensor("xT", (EMB, S), BF, kind="ExternalInput")
    wq = nc.dram_tensor("wq", (EMB, HQ * HD), BF, kind="ExternalInput")
    wk = nc.dram_tensor("wk", (EMB, HKV * HD), BF, kind="ExternalInput")
    wv = nc.dram_tensor("wv", (EMB, HKV * HD), BF, kind="ExternalInput")
    wo = nc.dram_tensor("wo", (HQ * HD, EMB), BF, kind="ExternalInput")
    cosT = nc.dram_tensor("cosT", (HD, S), F32, kind="ExternalInput")
    sinT = nc.dram_tensor("sinT", (HD, S), F32, kind="ExternalInput")
    out = nc.dram_tensor("out", (S, EMB), BF, kind="ExternalOutput")

    with tile.TileContext(nc) as tc, ExitStack() as ctx:
        persist = ctx.enter_context(tc.tile_pool(name="persist", bufs=1))
        # qk_sb j-blocks: 0..3 = roped Q heads, 4..5 = roped K kv-heads; [d, s]
        qk_sb = persist.tile([128, HQ + HKV, S], BF)
        # V in [t, d] layout: [t_part, t_chunk, kvl*128+d]
        v_sb = persist.tile([128, SC16, HKV * HD], BF)
        ctx_sb = persist.tile([128, HQ, S], BF)      # [d, head, s]
        wo_sb = persist.tile([128, HQ, EMB], BF)     # [d, head, e_out]
        wq_sb = persist.tile([128, NE, HQ * HD], BF)
        wk_sb = persist.tile([128, NE, HKV * HD], BF)
        wv_sb = persist.tile([128, NE, HKV * HD], BF)
        cos_sb = persist.tile([128, S], F32)
        sin_sb = persist.tile([128, S], F32)

        def _body():
            # ---------------- prefix: projections + RoPE ----------------
            with tc.tile_pool(name="xt", bufs=1) as xt_pool, \
                 tc.tile_pool(name="ropet", bufs=4) as rope_t, \
                 tc.tile_pool(name="pproj", bufs=8, space=bass.MemorySpace.PSUM) as pp:
                xt_sb = xt_pool.tile([128, NE, S], BF)
                # DMA order: interleave wk chunks with x chunks so the c-th
                # K-proj matmul can start as soon as its pair lands; rope
                # tables next (needed ~30us in), then wv/wq/wo.
                for c in range(NE):
                    nc.sync.dma_start(out=wk_sb[:, c, :], in_=wk[c * 128:(c + 1) * 128, :])
                    nc.sync.dma_start(out=xt_sb[:, c, :], in_=xT[c * 128:(c + 1) * 128, :])
                for c in range(NE):
                    nc.sync.dma_start(out=wv_sb[:, c, :], in_=wv[c * 128:(c + 1) * 128, :])
                nc.sync.dma_start(out=cos_sb, in_=cosT[:, :])
                nc.sync.dma_start(out=sin_sb, in_=sinT[:, :])
                for c in range(NE):
                    nc.sync.dma_start(out=wq_sb[:, c, :], in_=wq[c * 128:(c + 1) * 128, :])
                for jb in range(HQ):
                    nc.sync.dma_start(out=wo_sb[:, jb, :], in_=wo[jb * 128:(jb + 1) * 128, :])

                def rope(pts, j, sl):
                    xs = rope_t.tile([128, 512], F32, tag="xs", name="xs")
                    nc.scalar.copy(xs, pts)
                    xw = rope_t.tile([128, 512], F32, tag="xw", name="xw")
                    nc.sync.dma_start(out=xw[0:64, :], in_=xs[64:128, :])
                    nc.sync.dma_start(out=xw[64:128, :], in_=xs[0:64, :])
                    t1 = rope_t.tile([128, 512], F32, tag="t1", name="t1")
                    nc.vector.tensor_mul(t1, xs, cos_sb[:, sl])
                    nc.vector.tensor_mul(xw, xw, sin_sb[:, sl])
                    nc.vector.tensor_add(qk_sb[:, j, sl], t1, xw)

                # K projection: c-outer across all 8 (kv, sc) tiles so PE
                # consumes x chunks in DMA arrival order.
                kts = [pp.tile([128, 512], F32, tag="pp", name=f"kp_{i}")
                       for i in range(2 * SC4)]
                for c in range(NE):
                    for kvl in range(HKV):
                        for sc in range(SC4):
                            nc.tensor.matmul(
                                kts[kvl * SC4 + sc],
                                wk_sb[:, c, kvl * 128:(kvl + 1) * 128],
                                xt_sb[:, c, sc * 512:(sc + 1) * 512],
                                start=(c == 0), stop=(c == NE - 1),
                            )
                for kvl in range(HKV):
                    for sc in range(SC4):
                        rope(kts[kvl * SC4 + sc], HQ + kvl,
                             slice(sc * 512, (sc + 1) * 512))

                # V projection in [t, d] layout (no rope)
                for st in range(SC16):
                    pv = pp.tile([128, 512], F32, tag="pp", name=f"pv_{st}")
                    for c in range(NE):
                        nc.tensor.matmul(
                            pv[:, 0:HKV * HD],
                            xt_sb[:, c, st * 128:(st + 1) * 128],
                            wv_sb[:, c, :],
                            start=(c == 0), stop=(c == NE - 1),
                        )
                    nc.scalar.copy(v_sb[:, st, :], pv[:, 0:HKV * HD])

                # Q projection + rope, per (head, sc)
                for jb in range(HQ):
                    for sc in range(SC4):
                        qt = pp.tile([128, 512], F32, tag="pp", name=f"qp_{jb}_{sc}")
                        for c in range(NE):
                            nc.tensor.matmul(
                                qt,
                                wq_sb[:, c, jb * 128:(jb + 1) * 128],
                                xt_sb[:, c, sc * 512:(sc + 1) * 512],
                                start=(c == 0), stop=(c == NE - 1),
                            )
                        rope(qt, jb, slice(sc * 512, (sc + 1) * 512))

            # ---------------- main: attention + out-proj, pipelined ------
            with tc.tile_pool(name="spsp", bufs=sps_bufs, space=bass.MemorySpace.PSUM) as psps, \
                 tc.tile_pool(name="cpsp", bufs=2, space=bass.MemorySpace.PSUM) as pcps, \
                 tc.tile_pool(name="poup", bufs=pou_bufs, space=bass.MemorySpace.PSUM) as ppou, \
                 tc.tile_pool(name="expool", bufs=18) as expool, \
                 tc.tile_pool(name="accp", bufs=1) as accp, \
                 tc.tile_pool(name="otp", bufs=4) as otp:

                def p3_task(sc, so, ec):
                    # out[s=128, e=512] for s-block so of window sc
                    pou = ppou.tile([128, 512], F32, tag="pou", name=f"pou_{sc}_{so}_{ec}")
                    sl = slice(sc * 512 + so * 128, sc * 512 + (so + 1) * 128)
                    for hh in range(HQ):
                        nc.tensor.matmul(
                            pou,
                            ctx_sb[:, hh, sl],
                            wo_sb[:, hh, ec * 512:(ec + 1) * 512],
                            start=(hh == 0), stop=(hh == HQ - 1),
                        )
                    ot = otp.tile([128, 512], BF, tag="ot", name=f"ot_{sc}_{so}_{ec}")
                    nc.vector.tensor_copy(ot, pou)
                    nc.sync.dma_start(
                        out=out[sc * 512 + so * 128: sc * 512 + (so + 1) * 128,
                                ec * 512:(ec + 1) * 512],
                        in_=ot,
                    )

                # out-proj tasks for window sc, spread over groups of window
                # sc+1 (ctx of window sc complete only after its last head).
                p3_sched = {}      # tasks interleaved into a group's slots
                p3_sched_end = {}  # tasks appended after a group's norm step
                for sc in range(SC4 - 1):
                    tasks = [(sc, so, ec) for so in range(4) for ec in range(4)]
                    g0 = 4 * (sc + 1)
                    # ctx of window sc is complete only after norm(g0-1),
                    # which is emitted inside group g0 — so g0's share goes
                    # at the end of that group.
                    p3_sched_end[g0] = tasks[0:4]
                    p3_sched[g0 + 1] = tasks[4:8]
                    p3_sched[g0 + 2] = tasks[8:12]
                    p3_sched[g0 + 3] = tasks[12:16]

                state = {}  # g -> dict(ex=[...], cps=tile, rc=tile, h, sc)

                def emit_group(g):
                    sc, h = divmod(g, HQ)
                    kvjb = HQ + h // 2
                    kvl = h // 2
                    ssl = slice(sc * 512, (sc + 1) * 512)
                    st = {"h": h, "sc": sc, "ex": [], "a1": []}
                    state[g] = st
                    prev = state.get(g - 1)
                    p3q = list(p3_sched.get(g, []))
                    if prev is not None:
                        prev["cps"] = pcps.tile([128, 512], F32, tag="cps",
                                                name=f"cps_{g-1}")
                    for p in range(NPAIR):
                        sps2 = psps.tile([128, 2, 512], F32, tag="sps",
                                         name=f"sps_{g}_{p}")
                        for j in range(2):
                            nc.tensor.matmul(
                                sps2[:, j, :],
                                qk_sb[:, kvjb, (2 * p + j) * 128:(2 * p + j + 1) * 128],
                                qk_sb[:, h, ssl],
                                start=True, stop=True,
                            )
                        ex2 = expool.tile([128, 2, 512], BF, tag="ex",
                                          name=f"ex_{g}_{p}")
                        nc.scalar.activation(
                            ex2, sps2, mybir.ActivationFunctionType.Exp, scale=SCALE
                        )
                        st["ex"].append(ex2)
                        if prev is not None:
                            pkvl = prev["h"] // 2
                            for j in range(2):
                                nc.tensor.matmul(
                                    prev["cps"],
                                    v_sb[:, 2 * p + j, pkvl * 128:(pkvl + 1) * 128],
                                    prev["ex"][p][:, j, :],
                                    start=(p == 0 and j == 0),
                                    stop=(p == NPAIR - 1 and j == 1),
                                )
                        if p % 2 == 1:
                            a1 = accp.tile([128, 2, 512], BF, tag="a1", bufs=6,
                                           name=f"a1_{g}_{p//2}")
                            nc.vector.tensor_add(a1, st["ex"][p - 1], st["ex"][p])
                            st["a1"].append(a1)
                        if p in (1, 3, 5, 7) and p3q:
                            p3_task(*p3q.pop(0))
                    while p3q:
                        p3_task(*p3q.pop(0))
                    # denominator: finish bf16 tree, partition all-reduce,
                    # reciprocal (broadcast across partitions comes free).
                    a2_0 = accp.tile([128, 2, 512], BF, tag="a2", bufs=3, name=f"a20_{g}")
                    nc.vector.tensor_add(a2_0, st["a1"][0], st["a1"][1])
                    a2_1 = accp.tile([128, 2, 512], BF, tag="a2", bufs=3, name=f"a21_{g}")
                    nc.vector.tensor_add(a2_1, st["a1"][2], st["a1"][3])
                    a3 = accp.tile([128, 2, 512], BF, tag="a3", bufs=2, name=f"a3_{g}")
                    nc.vector.tensor_add(a3, a2_0, a2_1)
                    fold = accp.tile([128, 512], F32, tag="fold", bufs=2, name=f"fold_{g}")
                    nc.vector.tensor_add(fold, a3[:, 0, :], a3[:, 1, :])
                    rb = accp.tile([128, 512], F32, tag="rb", bufs=2, name=f"rb_{g}")
                    nc.gpsimd.partition_all_reduce(rb, fold, 128, bass_isa.ReduceOp.add)
                    rc = accp.tile([128, 512], F32, tag="rc", bufs=3, name=f"rc_{g}")
                    nc.vector.reciprocal(rc, rb)
                    st["rc"] = rc
                    # normalize the previous group's context (its cps chain
                    # just finished inside this group)
                    if prev is not None:
                        psl = slice(prev["sc"] * 512, (prev["sc"] + 1) * 512)
                        nc.vector.tensor_mul(
                            ctx_sb[:, prev["h"], psl], prev["cps"], prev["rc"])
                        del prev["ex"]
                    for t in p3_sched_end.get(g, []):
                        p3_task(*t)

                for g in range(HQ * SC4):
                    emit_group(g)

                # drain: cps + norm of the last group, then out-proj of the
                # last window.
                last = state[HQ * SC4 - 1]
                lkvl = last["h"] // 2
                last["cps"] = pcps.tile([128, 512], F32, tag="cps", name="cps_last")
                for p in range(NPAIR):
                    for j in range(2):
                        nc.tensor.matmul(
                            last["cps"],
                            v_sb[:, 2 * p + j, lkvl * 128:(lkvl + 1) * 128],
                            last["ex"][p][:, j, :],
                            start=(p == 0 and j == 0),
                            stop=(p == NPAIR - 1 and j == 1),
                        )
                lsl = slice(last["sc"] * 512, (last["sc"] + 1) * 512)
                nc.vector.tensor_mul(ctx_sb[:, last["h"], lsl], last["cps"], last["rc"])
                for so in range(4):
                    for ec in range(4):
                        p3_task(SC4 - 1, so, ec)

        if loop_n is not None:
            with tc.For_i(0, loop_n, 1):
                _body()
        else:
            _body()

    nc.compile()
    return nc


def _get_nc():
    global _NC
    if _NC is None:
        _NC = _build_program()
    return _NC


def _rope_tables():
    half = HD // 2
    inv_freq = 1.0 / (10000.0 ** (np.arange(half, dtype=np.float64) * 2.0 / HD))
    ang = np.arange(S, dtype=np.float64)[:, None] * inv_freq[None, :]  # (S, 64)
    cos = np.concatenate([np.cos(ang), np.cos(ang)], axis=1).T  # (128, S)
    sin = np.concatenate([-np.sin(ang), np.sin(ang)], axis=1).T  # pre-signed
    return (np.ascontiguousarray(cos, dtype=np.float32),
            np.ascontiguousarray(sin, dtype=np.float32))


def build_in_maps(x, W_Q, W_K, W_V, W_O):
    x = np.asarray(x, dtype=np.float32)
    W_Q = np.asarray(W_Q, dtype=np.float32)
    W_K = np.asarray(W_K, dtype=np.float32)
    W_V = np.asarray(W_V, dtype=np.float32)
    W_O = np.asarray(W_O, dtype=np.float32)
    cos, sin = _rope_tables()
    in_maps = []
    xTb = [np.ascontiguousarray(x[b].T).astype(bfnp) for b in range(B)]
    for b in range(B):
        for t in range(TP):
            qheads = list(range(HQ * t, HQ * t + HQ))
            kvheads = [HKV * t + i for i in range(HKV)]
            idxq = [d * HEADS + h for h in qheads for d in range(HD)]
            idxkv = [d * KV + kv for kv in kvheads for d in range(HD)]
            rows_o = [h * HD + d for h in qheads for d in range(HD)]
            in_maps.append(dict(
                xT=xTb[b],
                wq=np.ascontiguousarray(W_Q[idxq, :].T).astype(bfnp),
                wk=np.ascontiguousarray(W_K[idxkv, :].T).astype(bfnp),
                wv=np.ascontiguousarray(W_V[idxkv, :].T).astype(bfnp),
                wo=np.ascontiguousarray(W_O[:, rows_o].T).astype(bfnp),
                cosT=cos,
                sinT=sin,
            ))
    return in_maps


def combine_outs(outs):
    out = np.empty((B, S, EMB), dtype=np.float32)
    for b in range(B):
        acc = np.asarray(outs[TP * b], dtype=np.float32).copy()
        for t in range(1, TP):
            acc += np.asarray(outs[TP * b + t], dtype=np.float32)
        out[b] = acc
    return out


LAST_RESULTS = None


def kernel(x, W_Q, W_K, W_V, W_O):
    global LAST_RESULTS
    from concourse.bass_utils import run_bass_kernel_spmd

    nc = _get_nc()
    in_maps = build_in_maps(x, W_Q, W_K, W_V, W_O)
    res = run_bass_kernel_spmd(nc, in_maps, list(range(NCORES)))
    LAST_RESULTS = res
    outs = [r["out"] for r in res.results]
    return combine_outs(outs)


# revision 39
# speedup vs baseline: 10.1642x; 1.0051x over previous
# GQA attention block on 8 Trainium2 NeuronCores.
# Sharding: core = (batch b in {0,1}) x (tensor-parallel t in {0..3}).
# Each core: batch row b, 4 query heads {4t..4t+3}, 2 kv heads {2t, 2t+1}.
# W_Q/W_K/W_V split column-wise (per-head), W_O row-wise; the 4 TP partial
# outputs per batch are summed on the host (the "all-reduce").
#
# Schedule (per core): prefix = projections + RoPE (PE-dense, DMA-ordered so
# K-proj starts ~4us in); main = 16 groups (sc-outer, h-inner) software-
# pipelined one group deep: sps(g) | exp(g) on ACT | cps(g-1) | out-proj
# filler of window sc-1; softmax denominator via DVE bf16 tree + gpsimd
# partition_all_reduce (no PE matmuls wasted on the ones-reduction).
import math
import sys

sys.path.insert(0, "/opt/trn_rl_repo")

import ml_dtypes
import numpy as np

import concourse.bacc as bacc
import concourse.bass as bass
import concourse.bass_isa as bass_isa
import concourse.mybir as mybir
import concourse.tile as tile
from contextlib import ExitStack

BF = mybir.dt.bfloat16
F32 = mybir.dt.float32
bfnp = ml_dtypes.bfloat16

EMB = 2048
HEADS = 16
G = 2
HD = 128          # head dim
KV = HEADS // G   # 8 kv heads
B = 2
S = 2048
NCORES = 8
TP = 4
HQ = HEADS // TP       # 4 q heads per core
HKV = KV // TP         # 2 kv heads per core
NE = EMB // 128        # 16 contraction chunks
SC4 = S // 512         # 4 s-chunks of 512
SC16 = S // 128        # 16 s-chunks of 128
NPAIR = SC16 // 2      # 8 t-chunk pairs in attention
SCALE = 1.0 / math.sqrt(float(EMB))

_NC = None


def _build_program(loop_n=None, sps_bufs=2, pou_bufs=2):
    nc = bacc.Bacc("TRN2", target_bir_lowering=False, debug=False)

    xT = nc.dram_tensor("xT", (EMB, S), BF, kind="ExternalInput")
    wq = nc.dram_tensor("wq", (EMB, HQ * HD), BF, kind="ExternalInput")
    wk = nc.dram_tensor("wk", (EMB, HKV * HD), BF, kind="ExternalInput")
    wv = nc.dram_tensor("wv", (EMB, HKV * HD), BF, kind="ExternalInput")
    wo = nc.dram_tensor("wo", (HQ * HD, EMB), BF, kind="ExternalInput")
    cosT = nc.dram_tensor("cosT", (HD, S), F32, kind="ExternalInput")
    sinT = nc.dram_tensor("sinT", (HD, S), F32, kind="ExternalInput")
    out = nc.dram_tensor("out", (S, EMB), BF, kind="ExternalOutput")

    with tile.TileContext(nc) as tc, ExitStack() as ctx:
        persist = ctx.enter_context(tc.tile_pool(name="persist", bufs=1))
        # qk_sb j-blocks: 0..3 = roped Q heads, 4..5 = roped K kv-heads; [d, s]
        qk_sb = persist.tile([128, HQ + HKV, S], BF)
        # V in [t, d] layout: [t_part, t_chunk, kvl*128+d]
        v_sb = persist.tile([128, SC16, HKV * HD], BF)
        ctx_sb = persist.tile([128, HQ, S], BF)      # [d, head, s]
        wo_sb = persist.tile([128, HQ, EMB], BF)     # [d, head, e_out]
        wq_sb = persist.tile([128, NE, HQ * HD], BF)
        wk_sb = persist.tile([128, NE, HKV * HD], BF)
        wv_sb = persist.tile([128, NE, HKV * HD], BF)
        cos_sb = persist.tile([128, S], F32)
        sin_sb = persist.tile([128, S], F32)

        def _body():
            # ---------------- prefix: projections + RoPE ----------------
            with tc.tile_pool(name="xt", bufs=1) as xt_pool, \
                 tc.tile_pool(name="ropet", bufs=4) as rope_t, \
                 tc.tile_pool(name="pproj", bufs=8, space=bass.MemorySpace.PSUM) as pp:
                xt_sb = xt_pool.tile([128, NE, S], BF)
                # DMA order: interleave wk chunks with x chunks so the c-th
                # K-proj matmul can start as soon as its pair lands; rope
                # tables next (needed ~30us in), then wv/wq/wo.
                for c in range(NE):
                    nc.sync.dma_start(out=wk_sb[:, c, :], in_=wk[c * 128:(c + 1) * 128, :])
                    nc.sync.dma_start(out=xt_sb[:, c, :], in_=xT[c * 128:(c + 1) * 128, :])
                for c in range(NE):
                    nc.sync.dma_start(out=wv_sb[:, c, :], in_=wv[c * 128:(c + 1) * 128, :])
                nc.sync.dma_start(out=cos_sb, in_=cosT[:, :])
                nc.sync.dma_start(out=sin_sb, in_=sinT[:, :])
                for c in range(NE):
                    nc.sync.dma_start(out=wq_sb[:, c, :], in_=wq[c * 128:(c + 1) * 128, :])
                for jb in range(HQ):
                    nc.sync.dma_start(out=wo_sb[:, jb, :], in_=wo[jb * 128:(jb + 1) * 128, :])

                # Warm the PE during the initial DMA wait: dummy matmuls on
                # resident zeros start the p-state ramp (0.65->2.4GHz takes
                # ~3us of continuous busy) so the first real matmuls run at
                # full clock.
                wz = rope_t.tile([128, 128], BF, tag="wz", bufs=1, name="wz")
                nc.vector.memset(wz, 0.0)
                warm = pp.tile([128, 512], F32, tag="pp", name="warm")
                for i in range(40):
                    nc.tensor.matmul(warm[:, 0:64], wz, wz[:, 0:64],
                                     start=(i == 0), stop=(i == 39))

                def rope(pts, j, sl):
                    xs = rope_t.tile([128, 512], F32, tag="xs", name="xs")
                    nc.scalar.copy(xs, pts)
                    xw = rope_t.tile([128, 512], F32, tag="xw", name="xw")
                    nc.sync.dma_start(out=xw[0:64, :], in_=xs[64:128, :])
                    nc.sync.dma_start(out=xw[64:128, :], in_=xs[0:64, :])
                    t1 = rope_t.tile([128, 512], F32, tag="t1", name="t1")
                    nc.vector.tensor_mul(t1, xs, cos_sb[:, sl])
                    nc.vector.tensor_mul(xw, xw, sin_sb[:, sl])
                    nc.vector.tensor_add(qk_sb[:, j, sl], t1, xw)

                # K projection: c-outer across all 8 (kv, sc) tiles so PE
                # consumes x chunks in DMA arrival order.
                kts = [pp.tile([128, 512], F32, tag="pp", name=f"kp_{i}")
                       for i in range(2 * SC4)]
                for c in range(NE):
                    for kvl in range(HKV):
                        for sc in range(SC4):
                            nc.tensor.matmul(
                                kts[kvl * SC4 + sc],
                                wk_sb[:, c, kvl * 128:(kvl + 1) * 128],
                                xt_sb[:, c, sc * 512:(sc + 1) * 512],
                                start=(c == 0), stop=(c == NE - 1),
                            )
                for kvl in range(HKV):
                    for sc in range(SC4):
                        rope(kts[kvl * SC4 + sc], HQ + kvl,
                             slice(sc * 512, (sc + 1) * 512))

                # V projection in [t, d] layout (no rope)
                for st in range(SC16):
                    pv = pp.tile([128, 512], F32, tag="pp", name=f"pv_{st}")
                    for c in range(NE):
                        nc.tensor.matmul(
                            pv[:, 0:HKV * HD],
                            xt_sb[:, c, st * 128:(st + 1) * 128],
                            wv_sb[:, c, :],
                            start=(c == 0), stop=(c == NE - 1),
                        )
                    nc.scalar.copy(v_sb[:, st, :], pv[:, 0:HKV * HD])

                # Q projection + rope, per (head, sc)
                for jb in range(HQ):
                    for sc in range(SC4):
                        qt = pp.tile([128, 512], F32, tag="pp", name=f"qp_{jb}_{sc}")
                        for c in range(NE):
                            nc.tensor.matmul(
                                qt,
                                wq_sb[:, c, jb * 128:(jb + 1) * 128],
                                xt_sb[:, c, sc * 512:(sc + 1) * 512],
                                start=(c == 0), stop=(c == NE - 1),
                            )
                        rope(qt, jb, slice(sc * 512, (sc + 1) * 512))

            # ---------------- main: attention + out-proj, pipelined ------
            with tc.tile_pool(name="spsp", bufs=sps_bufs, space=bass.MemorySpace.PSUM) as psps, \
                 tc.tile_pool(name="cpsp", bufs=2, space=bass.MemorySpace.PSUM) as pcps, \
                 tc.tile_pool(name="poup", bufs=pou_bufs, space=bass.MemorySpace.PSUM) as ppou, \
                 tc.tile_pool(name="expool", bufs=18) as expool, \
                 tc.tile_pool(name="accp", bufs=1) as accp, \
                 tc.tile_pool(name="otp", bufs=4) as otp:

                def p3_half(holder, sc, so, ec, half):
                    # half 0: allocate pou + first 2 head-chunks; half 1:
                    # last 2 chunks + copy + store. Splitting lets the task
                    # spread across two pipeline slots.
                    sl = slice(sc * 512 + so * 128, sc * 512 + (so + 1) * 128)
                    if half == 0:
                        holder["pou"] = ppou.tile([128, 512], F32, tag="pou",
                                                  name=f"pou_{sc}_{so}_{ec}")
                    for hh in (0, 1) if half == 0 else (2, 3):
                        nc.tensor.matmul(
                            holder["pou"],
                            ctx_sb[:, hh, sl],
                            wo_sb[:, hh, ec * 512:(ec + 1) * 512],
                            start=(hh == 0), stop=(hh == HQ - 1),
                        )
                    if half == 1:
                        ot = otp.tile([128, 512], BF, tag="ot", name=f"ot_{sc}_{so}_{ec}")
                        nc.vector.tensor_copy(ot, holder["pou"])
                        nc.sync.dma_start(
                            out=out[sc * 512 + so * 128: sc * 512 + (so + 1) * 128,
                                    ec * 512:(ec + 1) * 512],
                            in_=ot,
                        )

                def p3_task(sc, so, ec):
                    holder = {}
                    p3_half(holder, sc, so, ec, 0)
                    p3_half(holder, sc, so, ec, 1)

                # out-proj tasks for window sc, spread over groups of window
                # sc+1 (ctx of window sc complete only after its last head).
                p3_sched = {}      # tasks interleaved into a group's slots
                p3_sched_end = {}  # tasks appended after a group's norm step
                for sc in range(SC4 - 1):
                    tasks = [(sc, so, ec) for so in range(4) for ec in range(4)]
                    g0 = 4 * (sc + 1)
                    # ctx of window sc is complete only after norm(g0-1),
                    # which is emitted inside group g0 — so g0's share goes
                    # at the end of that group.
                    p3_sched_end[g0] = tasks[0:4]
                    p3_sched[g0 + 1] = tasks[4:8]
                    p3_sched[g0 + 2] = tasks[8:12]
                    p3_sched[g0 + 3] = tasks[12:16]

                state = {}  # g -> dict(ex=[...], cps=tile, rc=tile, h, sc)

                def emit_group(g):
                    sc, h = divmod(g, HQ)
                    kvjb = HQ + h // 2
                    kvl = h // 2
                    ssl = slice(sc * 512, (sc + 1) * 512)
                    st = {"h": h, "sc": sc, "ex": [], "a1": []}
                    state[g] = st
                    prev = state.get(g - 1)
                    # flatten this group's out-proj tasks into half-task
                    # quanta (2 matmuls each) so every slot gets filler
                    p3q = []
                    for t in p3_sched.get(g, []):
                        holder = {}
                        p3q.append((holder, t, 0))
                        p3q.append((holder, t, 1))
                    if prev is not None:
                        prev["cps"] = pcps.tile([128, 512], F32, tag="cps",
                                                name=f"cps_{g-1}")
                    for p in range(NPAIR):
                        sps2 = psps.tile([128, 2, 512], F32, tag="sps",
                                         name=f"sps_{g}_{p}")
                        for j in range(2):
                            nc.tensor.matmul(
                                sps2[:, j, :],
                                qk_sb[:, kvjb, (2 * p + j) * 128:(2 * p + j + 1) * 128],
                                qk_sb[:, h, ssl],
                                start=True, stop=True,
                            )
                        ex2 = expool.tile([128, 2, 512], BF, tag="ex",
                                          name=f"ex_{g}_{p}")
                        nc.scalar.activation(
                            ex2, sps2, mybir.ActivationFunctionType.Exp, scale=SCALE
                        )
                        st["ex"].append(ex2)
                        if prev is not None:
                            pkvl = prev["h"] // 2
                            for j in range(2):
                                nc.tensor.matmul(
                                    prev["cps"],
                                    v_sb[:, 2 * p + j, pkvl * 128:(pkvl + 1) * 128],
                                    prev["ex"][p][:, j, :],
                                    start=(p == 0 and j == 0),
                                    stop=(p == NPAIR - 1 and j == 1),
                                )
                        if p % 2 == 1:
                            a1 = accp.tile([128, 2, 512], BF, tag="a1", bufs=6,
                                           name=f"a1_{g}_{p//2}")
                            nc.vector.tensor_add(a1, st["ex"][p - 1], st["ex"][p])
                            st["a1"].append(a1)
                        if p3q:
                            holder, t, half = p3q.pop(0)
                            p3_half(holder, *t, half)
                    while p3q:
                        holder, t, half = p3q.pop(0)
                        p3_half(holder, *t, half)
                    # denominator: finish bf16 tree, partition all-reduce,
                    # reciprocal (broadcast across partitions comes free).
                    a2_0 = accp.tile([128, 2, 512], BF, tag="a2", bufs=3, name=f"a20_{g}")
                    nc.vector.tensor_add(a2_0, st["a1"][0], st["a1"][1])
                    a2_1 = accp.tile([128, 2, 512], BF, tag="a2", bufs=3, name=f"a21_{g}")
                    nc.vector.tensor_add(a2_1, st["a1"][2], st["a1"][3])
                    a3 = accp.tile([128, 2, 512], BF, tag="a3", bufs=2, name=f"a3_{g}")
                    nc.vector.tensor_add(a3, a2_0, a2_1)
                    fold = accp.tile([128, 512], F32, tag="fold", bufs=2, name=f"fold_{g}")
                    nc.vector.tensor_add(fold, a3[:, 0, :], a3[:, 1, :])
                    rb = accp.tile([128, 512], F32, tag="rb", bufs=2, name=f"rb_{g}")
                    nc.gpsimd.partition_all_reduce(rb, fold, 128, bass_isa.ReduceOp.add)
                    rc = accp.tile([128, 512], F32, tag="rc", bufs=3, name=f"rc_{g}")
                    nc.vector.reciprocal(rc, rb)
                    st["rc"] = rc
                    # normalize the previous group's context (its cps chain
                    # just finished inside this group)
                    if prev is not None:
                        psl = slice(prev["sc"] * 512, (prev["sc"] + 1) * 512)
                        nc.vector.tensor_mul(
                            ctx_sb[:, prev["h"], psl], prev["cps"], prev["rc"])
                        del prev["ex"]
                    for t in p3_sched_end.get(g, []):
                        p3_task(*t)

                for g in range(HQ * SC4):
                    emit_group(g)

                # drain: cps + norm of the last group, then out-proj of the
                # last window.
                last = state[HQ * SC4 - 1]
                lkvl = last["h"] // 2
                last["cps"] = pcps.tile([128, 512], F32, tag="cps", name="cps_last")
                for p in range(NPAIR):
                    for j in range(2):
                        nc.tensor.matmul(
                            last["cps"],
                            v_sb[:, 2 * p + j, lkvl * 128:(lkvl + 1) * 128],
                            last["ex"][p][:, j, :],
                            start=(p == 0 and j == 0),
                            stop=(p == NPAIR - 1 and j == 1),
                        )
                lsl = slice(last["sc"] * 512, (last["sc"] + 1) * 512)
                nc.vector.tensor_mul(ctx_sb[:, last["h"], lsl], last["cps"], last["rc"])
                for so in range(4):
                    for ec in range(4):
                        p3_task(SC4 - 1, so, ec)

        if loop_n is not None:
            with tc.For_i(0, loop_n, 1):
                _body()
        else:
            _body()

    nc.compile()
    return nc


def _get_nc():
    global _NC
    if _NC is None:
        _NC = _build_program()
    return _NC


def _rope_tables():
    half = HD // 2
    inv_freq = 1.0 / (10000.0 ** (np.arange(half, dtype=np.float64) * 2.0 / HD))
    ang = np.arange(S, dtype=np.float64)[:, None] * inv_freq[None, :]  # (S, 64)
    cos = np.concatenate([np.cos(ang), np.cos(ang)], axis=1).T  # (128, S)
    sin = np.concatenate([-np.sin(ang), np.sin(ang)], axis=1).T  # pre-signed
    return (np.ascontiguousarray(cos, dtype=np.float32),
            np.ascontiguousarray(sin, dtype=np.float32))


def build_in_maps(x, W_Q, W_K, W_V, W_O):
    x = np.asarray(x, dtype=np.float32)
    W_Q = np.asarray(W_Q, dtype=np.float32)
    W_K = np.asarray(W_K, dtype=np.float32)
    W_V = np.asarray(W_V, dtype=np.float32)
    W_O = np.asarray(W_O, dtype=np.float32)
    cos, sin = _rope_tables()
    in_maps = []
    xTb = [np.ascontiguousarray(x[b].T).astype(bfnp) for b in range(B)]
    for b in range(B):
        for t in range(TP):
            qheads = list(range(HQ * t, HQ * t + HQ))
            kvheads = [HKV * t + i for i in range(HKV)]
            idxq = [d * HEADS + h for h in qheads for d in range(HD)]
            idxkv = [d * KV + kv for kv in kvheads for d in range(HD)]
            rows_o = [h * HD + d for h in qheads for d in range(HD)]
            in_maps.append(dict(
                xT=xTb[b],
                wq=np.ascontiguousarray(W_Q[idxq, :].T).astype(bfnp),
                wk=np.ascontiguousarray(W_K[idxkv, :].T).astype(bfnp),
                wv=np.ascontiguousarray(W_V[idxkv, :].T).astype(bfnp),
                wo=np.ascontiguousarray(W_O[:, rows_o].T).astype(bfnp),
                cosT=cos,
                sinT=sin,
            ))
    return in_maps


def combine_outs(outs):
    out = np.empty((B, S, EMB), dtype=np.float32)
    for b in range(B):
        acc = np.asarray(outs[TP * b], dtype=np.float32).copy()
        for t in range(1, TP):
            acc += np.asarray(outs[TP * b + t], dtype=np.float32)
        out[b] = acc
    return out


LAST_RESULTS = None


def kernel(x, W_Q, W_K, W_V, W_O):
    global LAST_RESULTS
    from concourse.bass_utils import run_bass_kernel_spmd

    nc = _get_nc()
    in_maps = build_in_maps(x, W_Q, W_K, W_V, W_O)
    res = run_bass_kernel_spmd(nc, in_maps, list(range(NCORES)))
    LAST_RESULTS = res
    outs = [r["out"] for r in res.results]
    return combine_outs(outs)
